# revision 46
# baseline (speedup 1.0000x reference)
"""Distributed causal-attention kernel for 8 Trainium2 NeuronCores.

Reference computation (B=2, T=2048, C=2048, H=16, hd=128):
  q,k,v = rope(x @ Wq.T), rope(x @ Wk.T), x @ Wv.T   (per-head)
  y = (softmax(q k^T / sqrt(hd) + mask) v, concat heads) @ Wo.T

Sharding: tensor-parallel over heads across all 8 cores (H/8 heads per
core, both batches processed on every core). Per-head attention runs in
the transposed layout (S^T = k_tile^T q_chunk) so the PV matmul needs
no transposes; softmax skips the max-subtraction (scores are bounded
here, exp stays in fp32 range) and gets its denominator via a
ones-vector matmul (partition-axis sum). A single 8-core AllToAll then
hands core (b*4+g) that head's outputs for batch b, t-slice g; head
h's A2A overlaps head h+1's attention, and in the tail the o_proj
contribution of already-gathered heads runs concurrently with the last
head's A2A (partials in SBUF). Matmuls run in float32r (full
PE rate; measured numerically identical to the fp32 matmul path on
TRN2).
"""
import sys

sys.path.insert(0, '/opt/trn_rl_repo')

import numpy as np
import concourse.bass as bass
import concourse.bacc as bacc
import concourse.mybir as mybir
import concourse.tile as tile
from concourse import bass_utils

F32 = mybir.dt.float32
F32R = mybir.dt.float32r
AF = mybir.ActivationFunctionType

ROPE_BASE = 10000.0
HD = 128           # head dim (C // n_heads)
B = 2              # batch (fixed: cores 0-3 <-> b=0, 4-7 <-> b=1)
N_CORES = 8


def _rope_tables(T):
    """Transposed RoPE tables [hd, T] plus the sign-folded sin table.

    q' = q * cosT + qswap * sinT_signed, where qswap is q with its
    partition halves swapped (no sign change):
      rows d < 64:  q'[d] = q[d] cos[d] - q[d+64] sin[d]
      rows d >= 64: q'[d] = q[d] cos[d] + q[d-64] sin[d]  (sin[d]=sin[d-64])
    """
    inv_freq = 1.0 / (ROPE_BASE ** (np.arange(0, HD, 2, dtype=np.float64) / HD))
    t = np.arange(T, dtype=np.float64)
    freqs = np.outer(t, inv_freq)                      # [T, hd/2]
    emb = np.concatenate([freqs, freqs], -1)           # [T, hd]
    cos = np.cos(emb).T.astype(np.float32)             # [hd, T]
    sin = np.sin(emb).T.astype(np.float32)
    sin_signed = sin.copy()
    sin_signed[:HD // 2] *= -1.0
    return cos, sin_signed


def _causal_binmask():
    """Paired diagonal masks [128, 2*1024]: variant v in {0,1} covers a
    k-tile PAIR at offsets (2v, 2v+1) relative to the 512-wide q-chunk:
    col u = 512*half + qq, keep iff qq >= kk + 128*(2v + half)."""
    kk = np.arange(128)[:, None]
    out = np.empty((128, 2048), np.float32)
    for v in range(2):
        for half in range(2):
            qq = np.arange(512)[None, :]
            out[:, v * 1024 + half * 512: v * 1024 + (half + 1) * 512] = \
                (qq >= kk + 128 * (2 * v + half))
    return out


def build_nc(T, C, mode="causal"):
    """Build the SPMD Bass program. All 8 cores run identical code;
    per-core behavior (which heads / which output slice) comes from the
    inputs and the AllToAll.

    mode: 'causal' (skip upper-triangle blocks, binary-mask diagonal),
          'full' (no masking), 'masked' (additive mask input).
    """
    HPC = C // HD // N_CORES     # heads per core
    D = HPC * HD                 # local channel count
    NCT = C // 128               # contraction tiles over C
    TO = T // 4                  # output t-slice width per core
    NQC = T // 512               # 512-wide q chunks per batch
    TCH = 256                    # projection t chunk
    NCH = B * T // TCH           # projection chunks (both batches)
    NTT = T // 128               # k/t tiles per batch
    NPR = NCT // HPC             # o_proj contraction tiles per local head
    DW = 256                     # o_proj d-chunk width
    NDJ = C // DW
    scale = 1.0 / np.sqrt(HD)

    nc = bacc.Bacc("TRN2", target_bir_lowering=False, debug=False,
                   num_devices=N_CORES)

    # host pre-tiled operands: every DMA below is fully contiguous.
    # xTt: chunk ch at rows [128ch,128ch+128) holds x^T[:, ch-chunk]
    # laid out as [128, NCT*TCH] (c-tile-major columns).
    xTt = nc.dram_tensor("xTt", [NCH * 128, NCT * TCH], F32R,
                         kind="ExternalInput")
    wqT = nc.dram_tensor("wqT", [128, NCT * D], F32R, kind="ExternalInput")
    wkT = nc.dram_tensor("wkT", [128, NCT * D], F32R, kind="ExternalInput")
    wvT = nc.dram_tensor("wvT", [128, NCT * D], F32R, kind="ExternalInput")
    # woTt: block (hh*NDJ + dj) at rows [.*128, .*128+128) = the o_proj
    # weight slice for local head hh, d-chunk dj, as [128, NPR*DW]
    woTt = nc.dram_tensor("woTt", [HPC * NDJ * 128, NPR * DW], F32R,
                          kind="ExternalInput")
    cosT = nc.dram_tensor("cosT", [HD, T], F32, kind="ExternalInput")
    sinT = nc.dram_tensor("sinT", [HD, T], F32, kind="ExternalInput")
    ones_in = nc.dram_tensor("ones_in", [128, 1], F32R, kind="ExternalInput")
    onesr_in = nc.dram_tensor("onesr_in", [1, 128], F32R, kind="ExternalInput")
    if mode == "causal":
        bmask = nc.dram_tensor("bmask", [128, 2048], F32, kind="ExternalInput")
    elif mode == "masked":
        maskT = nc.dram_tensor("maskT", [T, T], F32, kind="ExternalInput")
    y = nc.dram_tensor("y", [TO, C], F32, kind="ExternalOutput")

    def ktmax(qj):  # number of k-tiles for q-chunk qj
        return 4 * qj + 4 if mode == "causal" else NTT

    def wo_part_ap(hh, dj):
        r = (hh * NDJ + dj) * 128
        return woTt[r:r + 128, :]

    with tile.TileContext(nc) as tc, \
         tc.tile_pool(name="consts", bufs=1) as pc, \
         tc.tile_pool(name="dram", bufs=1, space="DRAM") as dram:
        ones_sb = pc.tile([128, 1], F32R)
        onesr_sb = pc.tile([1, 128], F32R)
        if mode == "causal":
            bm_sb = pc.tile([128, 2048], F32)

        def load_consts():
            nc.sync.dma_start(ones_sb[:], ones_in[:])
            nc.sync.dma_start(onesr_sb[:], onesr_in[:])
            if mode == "causal":
                nc.sync.dma_start(bm_sb[:], bmask[:])

        # AllToAll buffers, one pair per local head h: shard j (rows
        # j*HD..(j+1)*HD) holds this core's head-h outputs for dest
        # core j = (b*4 + t-slice). After A2A, cc_out[h] row block i =
        # core i's head h (channels i*D + h*HD ..), for THIS core's
        # (batch, t-slice). Per-head tensors let head h's A2A overlap
        # head h+1's attention.
        cc_in = [dram.tile([N_CORES * HD, TO], F32R, name=f"cc_in{h}")
                 for h in range(HPC)]
        cc_out = [dram.tile([N_CORES * HD, TO], F32R, name=f"cc_out{h}")
                  for h in range(HPC)]

        # qkv pool lives through attention, freed before o_proj
        with tc.tile_pool(name="qkv", bufs=1) as pq:
            # per (batch, head) pair p = b*HPC + h: q^T,k^T [hd, T]
            qT_sb = pq.tile([128, B * HPC * T], F32R)
            kT_sb = pq.tile([128, B * HPC * T], F32R)
            # v natural layout per batch: [T,128]-tile x [128, D]
            v_sb = pq.tile([128, B * NTT * D], F32R)

            # ============ Phase A: q,k,v projections + RoPE ============
            with tc.tile_pool(name="prj_w", bufs=1) as pw, \
                 tc.tile_pool(name="prj_x", bufs=2) as px, \
                 tc.tile_pool(name="prj_cs", bufs=2) as pcs, \
                 tc.tile_pool(name="prj_ps", bufs=8, space="PSUM") as pps, \
                 tc.tile_pool(name="prj_tmp", bufs=14) as pt:
                wq_sb = pw.tile([128, NCT * D], F32R)
                wk_sb = pw.tile([128, NCT * D], F32R)
                wv_sb = pw.tile([128, NCT * D], F32R)

                def load_w(wsb, wdr):
                    nc.sync.dma_start(wsb[:], wdr[:])

                def load_chunk(ch):
                    xch = px.tile([128, NCT * TCH], F32R, tag="xch",
                                  name="xch")
                    nc.sync.dma_start(
                        xch[:], xTt[ch * 128:(ch + 1) * 128, :])
                    tl = (ch * TCH) % T
                    cs = pcs.tile([128, TCH], F32, tag="cos", name="cs")
                    sn = pcs.tile([128, TCH], F32, tag="sin", name="sn")
                    nc.sync.dma_start(cs[:], cosT[:, tl:tl + TCH])
                    nc.sync.dma_start(sn[:], sinT[:, tl:tl + TCH])
                    return xch, cs, sn

                # interleave quarters of Wq and chunk-0 so the first
                # matmuls start after ~1MB instead of 4MB
                xch0 = px.tile([128, NCT * TCH], F32R, tag="xch",
                               name="xch0")
                QW = NCT * D // 4
                QX = NCT * TCH // 4
                for qtr in range(4):
                    nc.sync.dma_start(wq_sb[:, qtr * QW:(qtr + 1) * QW],
                                      wqT[:, qtr * QW:(qtr + 1) * QW])
                    nc.sync.dma_start(xch0[:, qtr * QX:(qtr + 1) * QX],
                                      xTt[0:128, qtr * QX:(qtr + 1) * QX])
                cs0 = pcs.tile([128, TCH], F32, tag="cos", name="cs0")
                sn0 = pcs.tile([128, TCH], F32, tag="sin", name="sn0")
                nc.sync.dma_start(cs0[:], cosT[:, 0:TCH])
                nc.sync.dma_start(sn0[:], sinT[:, 0:TCH])
                pre = (xch0, cs0, sn0)
                load_w(wk_sb, wkT)
                load_w(wv_sb, wvT)
                load_consts()

                for ch in range(NCH):
                    b = ch // (T // TCH)
                    tloc = (ch * TCH) % T
                    xch, cs, sn = pre if ch == 0 else load_chunk(ch)
                    pre = None
                    evac = []
                    for h in range(HPC):
                        p = b * HPC + h
                        for wsb, dst in ((wq_sb, qT_sb), (wk_sb, kT_sb)):
                            ps = pps.tile([128, TCH], F32, tag="pAqk",
                                          name="psA")
                            for ct in range(NCT):
                                nc.tensor.matmul(
                                    ps[:],
                                    wsb[:, ct * D + h * HD:
                                        ct * D + (h + 1) * HD],
                                    xch[:, ct * TCH:(ct + 1) * TCH],
                                    start=(ct == 0), stop=(ct == NCT - 1))
                            evac.append((ps, dst, p))
                    # pass 1: free the PSUM slots fast (t1 reads PSUM,
                    # the copy feeds the partition-swap DMA)
                    pass2 = []
                    for ps, dst, p in evac:
                        t1 = pt.tile([128, TCH], F32, tag="rope", name="t1")
                        nc.vector.tensor_mul(t1[:], ps[:], cs[:])
                        tmp = pt.tile([128, TCH], F32, tag="rope",
                                      name="rtmp")
                        nc.scalar.copy(tmp[:], ps[:])
                        sw = pt.tile([128, TCH], F32, tag="rope", name="rsw")
                        nc.gpsimd.dma_start(sw[0:64, :], tmp[64:128, :])
                        nc.gpsimd.dma_start(sw[64:128, :], tmp[0:64, :])
                        pass2.append((t1, sw, dst, p))
                    # v matmuls for this chunk (all local heads at once)
                    for st in range(TCH // 128):
                        tt = (ch * TCH) // 128 + st
                        ps = pps.tile([128, D], F32, tag="pAqk", name="psV")
                        for ct in range(NCT):
                            nc.tensor.matmul(
                                ps[:],
                                xch[:, ct * TCH + st * 128:
                                    ct * TCH + st * 128 + 128],
                                wv_sb[:, ct * D:(ct + 1) * D],
                                start=(ct == 0), stop=(ct == NCT - 1))
                        with nc.allow_low_precision(reason="f32r v evac"):
                            nc.scalar.copy(v_sb[:, tt * D:(tt + 1) * D],
                                           ps[:])
                    # pass 2: finish RoPE
                    for t1, sw, dst, p in pass2:
                        t2 = pt.tile([128, TCH], F32, tag="rope", name="t2")
                        nc.vector.tensor_mul(t2[:], sw[:], sn[:])
                        sl = dst[:, p * T + tloc: p * T + tloc + TCH]
                        with nc.allow_low_precision(reason="f32r rope"):
                            nc.vector.tensor_add(sl, t1[:], t2[:])

            # ============ Attention per (batch, head) =================
            # Software-pipelined emission: the S^T matmul for k-tile
            # kt+2 is issued before the den/PV matmuls of k-tile kt, so
            # the PE keeps streaming while ACT(exp)/DVE(mask) catch up.
            with tc.tile_pool(name="att_es", bufs=8) as pes, \
                 tc.tile_pool(name="att_o", bufs=3) as po, \
                 tc.tile_pool(name="att_ps", bufs=4, space="PSUM") as pas, \
                 tc.tile_pool(name="att_acc", bufs=2, space="PSUM") as paa, \
                 tc.tile_pool(name="att_msk", bufs=4) as pmk:
                for h in range(HPC):
                    for b in range(B):
                        p = b * HPC + h
                        for qj in range(NQC):
                            qsl = qT_sb[:, p * T + qj * 512:
                                        p * T + qj * 512 + 512]
                            kmax = ktmax(qj)
                            ps_den = paa.tile([1, 512], F32, tag="den",
                                              bufs=1, name="psden")
                            ps_o = paa.tile([128, 512], F32, tag="pvacc",
                                            name="pso")

                            def s_pair(kt2):
                                # two k-tiles share one 2-bank psum so a
                                # single ACT pass exponentiates both
                                s2 = pas.tile([128, 1024], F32, tag="s2",
                                              bufs=2, name="s2")
                                for half in range(2):
                                    kt = kt2 + half
                                    if kt >= kmax:
                                        break
                                    nc.tensor.matmul(
                                        s2[:, half * 512:half * 512 + 512],
                                        kT_sb[:, p * T + kt * 128:
                                              p * T + kt * 128 + 128],
                                        qsl, start=True, stop=True)
                                return s2

                            npair = (kmax + 1) // 2
                            s_tiles = {}
                            for kk in range(min(2, npair)):
                                s_tiles[kk] = s_pair(2 * kk)
                            for pi in range(npair):
                                s2 = s_tiles.pop(pi)
                                e2 = pes.tile([128, 1024], F32R, tag="es",
                                              name="e2")
                                if mode == "masked":
                                    sm2 = pmk.tile([128, 1024], F32,
                                                   tag="sm", name="sm2")
                                    mt2 = pmk.tile([128, 1024], F32,
                                                   tag="mt", name="mt2")
                                    nw = min(kmax - 2 * pi, 2)
                                    nc.sync.dma_start(
                                        mt2[:, :nw * 512].rearrange(
                                            "p (n q) -> p n q", n=nw),
                                        maskT[2 * pi * 128:
                                              (2 * pi + nw) * 128,
                                              qj * 512:(qj + 1) * 512]
                                        .rearrange("(n p) q -> p n q",
                                                   p=128))
                                    if nw < 2:
                                        nc.vector.memset(
                                            mt2[:, nw * 512:], 0.0)
                                    nc.vector.tensor_add(sm2[:], s2[:],
                                                         mt2[:])
                                    esrc = sm2
                                else:
                                    esrc = s2
                                with nc.allow_low_precision(reason="exp"):
                                    nc.scalar.activation(
                                        e2[:], esrc[:], AF.Exp,
                                        scale=float(scale))
                                if pi + 2 < npair:
                                    s_tiles[pi + 2] = s_pair(2 * (pi + 2))
                                if mode == "causal" and 2 * pi >= 4 * qj:
                                    v = (2 * pi - 4 * qj) // 2
                                    em2 = pes.tile([128, 1024], F32R,
                                                   tag="esm", bufs=4,
                                                   name="em2")
                                    with nc.allow_low_precision(
                                            reason="mask"):
                                        nc.vector.tensor_mul(
                                            em2[:], e2.bitcast(F32),
                                            bm_sb[:, v * 1024:
                                                  v * 1024 + 1024])
                                    esel = em2
                                else:
                                    esel = e2
                                for half in range(2):
                                    kt = 2 * pi + half
                                    if kt >= kmax:
                                        break
                                    e_t = esel[:, half * 512:half * 512 + 512]
                                    nc.tensor.matmul(
                                        ps_den[:], ones_sb[:], e_t,
                                        start=(kt == 0),
                                        stop=(kt == kmax - 1))
                                    nc.tensor.matmul(
                                        ps_o[:],
                                        v_sb[:, (b * NTT + kt) * D + h * HD:
                                             (b * NTT + kt) * D
                                             + (h + 1) * HD],
                                        e_t,
                                        start=(kt == 0),
                                        stop=(kt == kmax - 1))
                            # softmax denominator -> reciprocal, spread
                            # over partitions so DVE RECIPROCAL is wide
                            if True:
                                dsb = po.tile([1, 512], F32, tag="dsb",
                                              name="dsb")
                                nc.vector.tensor_copy(dsb[:], ps_den[:])
                                drs = po.tile([128, 4], F32, tag="drs",
                                              name="drs")
                                nc.sync.dma_start(drs[:], dsb[:])
                                rrs = po.tile([128, 4], F32, tag="rrs",
                                              name="rrs")
                                nc.vector.reciprocal(rrs[:], drs[:])
                                rsb = po.tile([1, 512], F32, tag="rsb",
                                              name="rsb")
                                nc.sync.dma_start(rsb[:], rrs[:])
                            bc = po.tile([128, 512], F32, tag="bc",
                                         name="bc")
                            nc.gpsimd.partition_broadcast(bc[:], rsb[:])
                            o_sc = po.tile([128, 512], F32R, tag="osc",
                                           name="osc")
                            with nc.allow_low_precision(reason="scale"):
                                nc.vector.tensor_mul(o_sc[:], ps_o[:],
                                                     bc[:])
                            # scatter the 512-wide q-chunk into shards
                            w = min(512, TO)
                            for s in range(512 // w):
                                t0 = qj * 512 + s * w    # global t in batch
                                shard = b * 4 + t0 // TO
                                nc.sync.dma_start(
                                    cc_in[h][shard * HD:(shard + 1) * HD,
                                             t0 % TO: t0 % TO + w],
                                    o_sc[:, s * w:(s + 1) * w])
                    # head h complete on both batches -> its AllToAll can
                    # overlap head h+1's attention
                    nc.gpsimd.collective_compute(
                        "AllToAll", mybir.AluOpType.bypass,
                        replica_groups=[list(range(N_CORES))],
                        ins=[cc_in[h].opt()], outs=[cc_out[h].opt()])

        # ============ Phase C: o_proj ================================
        # Heads 0..HPC-2 were gathered during attention, so their
        # o_proj contribution runs concurrently with the LAST head's
        # AllToAll (the PE is otherwise idle there); partials live in
        # SBUF. The last head's part joins when its A2A lands.
        with tc.tile_pool(name="phC_cc", bufs=1) as pcc, \
             tc.tile_pool(name="phC_w", bufs=3) as pcw, \
             tc.tile_pool(name="phC_y", bufs=4) as pcy, \
             tc.tile_pool(name="phC_y0", bufs=1) as pcy0, \
             tc.tile_pool(name="phC_ps", bufs=4, space="PSUM") as pcps:
            y0_sb = (pcy0.tile([128, (TO // 128) * C], F32, name="y0sb")
                     if HPC > 1 else None)
            cc_sb = {}

            def part(hh, first, last):
                t = pcc.tile([128, N_CORES * TO], F32R, name=f"cc_sb{hh}")
                nc.sync.dma_start(
                    t.rearrange("p (n t) -> p n t", n=N_CORES),
                    cc_out[hh].opt().rearrange("(n p) t -> p n t", p=128))
                cc_sb[hh] = t
                for dj in range(NDJ):
                    wo_sb = pcw.tile([128, NPR * DW], F32R, tag="wo",
                                     name="wo")
                    nc.sync.dma_start(
                        wo_sb.rearrange("p (n d) -> p n d", n=NPR),
                        wo_part_ap(hh, dj))
                    for tt in range(TO // 128):
                        ps = pcps.tile([128, DW], F32, tag="pC", name="psC")
                        for i in range(NPR):
                            nc.tensor.matmul(
                                ps[:],
                                cc_sb[hh][:, i * TO + tt * 128:
                                          i * TO + tt * 128 + 128],
                                wo_sb[:, i * DW:(i + 1) * DW],
                                start=(i == 0), stop=(i == NPR - 1))
                        y0sl = (y0_sb[:, tt * C + dj * DW:
                                      tt * C + (dj + 1) * DW]
                                if y0_sb is not None else None)
                        if last:
                            yt = pcy.tile([128, DW], F32, tag="yt",
                                          name="yt")
                            if first:
                                nc.scalar.copy(yt[:], ps[:])
                            else:
                                nc.vector.tensor_add(yt[:], ps[:], y0sl)
                            nc.sync.dma_start(
                                y[tt * 128:(tt + 1) * 128,
                                  dj * DW:(dj + 1) * DW], yt[:])
                        elif first:
                            nc.scalar.copy(y0sl, ps[:])
                        else:
                            nc.vector.tensor_add(y0sl, ps[:], y0sl)

            for hh in range(HPC):
                part(hh, first=(hh == 0), last=(hh == HPC - 1))

    nc.compile()
    return nc


_NC_CACHE = {}


def _get_nc(T, C, mode):
    key = (T, C, mode)
    if key not in _NC_CACHE:
        _NC_CACHE[key] = build_nc(T, C, mode)
    return _NC_CACHE[key]


def _detect_mode(mask):
    T = mask.shape[0]
    tri = np.tril(np.ones((T, T), dtype=bool))
    if not np.any(mask):
        return "full"
    if np.all(np.abs(mask[tri]) < 1e-6) and np.all(mask[~tri] < -1e8):
        return "causal"
    return "masked"


def kernel(x, mask, Wq, Wk, Wv, Wo):
    x = np.asarray(x)
    mask = np.asarray(mask)
    Bx, T, C = x.shape
    assert Bx == B
    HPC = C // HD // N_CORES
    TO = T // 4
    mode = _detect_mode(mask)
    nc = _get_nc(T, C, mode)

    cos, sin_signed = _rope_tables(T)
    NCT = C // 128
    TCH = 256
    NCH = B * T // TCH
    DW = 256
    NDJ = C // DW
    NPR = NCT // HPC
    D = HPC * HD
    xT2 = np.concatenate([x[0].T, x[1].T], axis=1)          # [C, B*T]
    # pre-tile x: [NCH*128, NCT*TCH]
    xtt = (xT2.reshape(NCT, 128, NCH, TCH).transpose(2, 1, 0, 3)
           .reshape(NCH * 128, NCT * TCH))
    xtt = np.ascontiguousarray(xtt)

    def tile_w(Wslice):     # [C, D] -> [128, NCT*D]
        wt = Wslice.T.reshape(NCT, 128, -1).transpose(1, 0, 2)
        return np.ascontiguousarray(wt.reshape(128, -1))

    WoT = np.asarray(Wo).T                                   # [C, C]
    # pre-tile wo: rows (hh*NDJ+dj)*128.. hold [128, NPR*DW] for the
    # NPR channel-tiles (i*HPC+hh) of d-chunk dj
    wott = np.empty((HPC * NDJ * 128, NPR * DW), np.float32)
    for hh in range(HPC):
        for dj in range(NDJ):
            blk = WoT.reshape(NPR, HPC, 128, C)[:, hh, :,
                                                dj * DW:(dj + 1) * DW]
            wott[(hh * NDJ + dj) * 128:(hh * NDJ + dj + 1) * 128, :] = \
                blk.transpose(1, 0, 2).reshape(128, NPR * DW)

    in_maps = []
    for core in range(N_CORES):
        hsl = slice(core * HPC * HD, (core + 1) * HPC * HD)
        m = {
            "xTt": xtt,
            "wqT": tile_w(np.asarray(Wq)[hsl, :]),
            "wkT": tile_w(np.asarray(Wk)[hsl, :]),
            "wvT": tile_w(np.asarray(Wv)[hsl, :]),
            "woTt": wott,
            "cosT": cos, "sinT": sin_signed,
            "ones_in": np.ones((128, 1), np.float32),
            "onesr_in": np.ones((1, 128), np.float32),
        }
        if mode == "causal":
            m["bmask"] = _causal_binmask()
        elif mode == "masked":
            m["maskT"] = np.ascontiguousarray(mask.T) * np.float32(np.sqrt(HD))
        in_maps.append(m)

    res = bass_utils.run_bass_kernel_spmd(nc, in_maps,
                                          core_ids=list(range(N_CORES)))

    out = np.empty((B, T, C), np.float32)
    for core in range(N_CORES):
        b, g = divmod(core, 4)
        out[b, g * TO:(g + 1) * TO, :] = res.results[core]["y"]
    return out


# revision 47
# speedup vs baseline: 1.0333x; 1.0333x over previous
"""Distributed causal-attention kernel for 8 Trainium2 NeuronCores.

Reference computation (B=2, T=2048, C=2048, H=16, hd=128):
  q,k,v = rope(x @ Wq.T), rope(x @ Wk.T), x @ Wv.T   (per-head)
  y = (softmax(q k^T / sqrt(hd) + mask) v, concat heads) @ Wo.T

Sharding: tensor-parallel over heads across all 8 cores (H/8 heads per
core, both batches processed on every core). Per-head attention runs in
the transposed layout (S^T = k_tile^T q_chunk) so the PV matmul needs
no transposes; softmax skips the max-subtraction (scores are bounded
here, exp stays in fp32 range) and gets its denominator via a
ones-vector matmul (partition-axis sum). A single 8-core AllToAll then
hands core (b*4+g) that head's outputs for batch b, t-slice g; head
h's A2A overlaps head h+1's attention, and in the tail the o_proj
contribution of already-gathered heads runs concurrently with the last
head's A2A (partials in SBUF). Matmuls run in float32r (full
PE rate; measured numerically identical to the fp32 matmul path on
TRN2).
"""
import sys

sys.path.insert(0, '/opt/trn_rl_repo')

import numpy as np
import concourse.bass as bass
import concourse.bacc as bacc
import concourse.mybir as mybir
import concourse.tile as tile
from concourse import bass_utils

F32 = mybir.dt.float32
F32R = mybir.dt.float32r
AF = mybir.ActivationFunctionType

ROPE_BASE = 10000.0
HD = 128           # head dim (C // n_heads)
B = 2              # batch (fixed: cores 0-3 <-> b=0, 4-7 <-> b=1)
N_CORES = 8


def _rope_tables(T):
    """Transposed RoPE tables [hd, T] plus the sign-folded sin table.

    q' = q * cosT + qswap * sinT_signed, where qswap is q with its
    partition halves swapped (no sign change):
      rows d < 64:  q'[d] = q[d] cos[d] - q[d+64] sin[d]
      rows d >= 64: q'[d] = q[d] cos[d] + q[d-64] sin[d]  (sin[d]=sin[d-64])
    """
    inv_freq = 1.0 / (ROPE_BASE ** (np.arange(0, HD, 2, dtype=np.float64) / HD))
    t = np.arange(T, dtype=np.float64)
    freqs = np.outer(t, inv_freq)                      # [T, hd/2]
    emb = np.concatenate([freqs, freqs], -1)           # [T, hd]
    cos = np.cos(emb).T.astype(np.float32)             # [hd, T]
    sin = np.sin(emb).T.astype(np.float32)
    sin_signed = sin.copy()
    sin_signed[:HD // 2] *= -1.0
    return cos, sin_signed


def _causal_binmask():
    """Paired diagonal masks [128, 2*1024]: variant v in {0,1} covers a
    k-tile PAIR at offsets (2v, 2v+1) relative to the 512-wide q-chunk:
    col u = 512*half + qq, keep iff qq >= kk + 128*(2v + half)."""
    kk = np.arange(128)[:, None]
    out = np.empty((128, 2048), np.float32)
    for v in range(2):
        for half in range(2):
            qq = np.arange(512)[None, :]
            out[:, v * 1024 + half * 512: v * 1024 + (half + 1) * 512] = \
                (qq >= kk + 128 * (2 * v + half))
    return out


def build_nc(T, C, mode="causal"):
    """Build the SPMD Bass program. All 8 cores run identical code;
    per-core behavior (which heads / which output slice) comes from the
    inputs and the AllToAll.

    mode: 'causal' (skip upper-triangle blocks, binary-mask diagonal),
          'full' (no masking), 'masked' (additive mask input).
    """
    HPC = C // HD // N_CORES     # heads per core
    D = HPC * HD                 # local channel count
    NCT = C // 128               # contraction tiles over C
    TO = T // 4                  # output t-slice width per core
    NQC = T // 512               # 512-wide q chunks per batch
    TCH = 256                    # projection t chunk
    NCH = B * T // TCH           # projection chunks (both batches)
    NTT = T // 128               # k/t tiles per batch
    NPR = NCT // HPC             # o_proj contraction tiles per local head
    DW = 256                     # o_proj d-chunk width
    NDJ = C // DW
    scale = 1.0 / np.sqrt(HD)

    nc = bacc.Bacc("TRN2", target_bir_lowering=False, debug=False,
                   num_devices=N_CORES)

    # host pre-tiled operands: every DMA below is fully contiguous.
    # xTt: chunk ch at rows [128ch,128ch+128) holds x^T[:, ch-chunk]
    # laid out as [128, NCT*TCH] (c-tile-major columns).
    xTt = nc.dram_tensor("xTt", [NCH * 128, NCT * TCH], F32R,
                         kind="ExternalInput")
    wqT = nc.dram_tensor("wqT", [128, NCT * D], F32R, kind="ExternalInput")
    wkT = nc.dram_tensor("wkT", [128, NCT * D], F32R, kind="ExternalInput")
    wvT = nc.dram_tensor("wvT", [128, NCT * D], F32R, kind="ExternalInput")
    # woTt: block (hh*NDJ + dj) at rows [.*128, .*128+128) = the o_proj
    # weight slice for local head hh, d-chunk dj, as [128, NPR*DW]
    woTt = nc.dram_tensor("woTt", [HPC * NDJ * 128, NPR * DW], F32R,
                          kind="ExternalInput")
    cosT = nc.dram_tensor("cosT", [HD, T], F32, kind="ExternalInput")
    sinT = nc.dram_tensor("sinT", [HD, T], F32, kind="ExternalInput")
    ones_in = nc.dram_tensor("ones_in", [128, 1], F32R, kind="ExternalInput")
    onesr_in = nc.dram_tensor("onesr_in", [1, 128], F32R, kind="ExternalInput")
    if mode == "causal":
        bmask = nc.dram_tensor("bmask", [128, 2048], F32, kind="ExternalInput")
    elif mode == "masked":
        maskT = nc.dram_tensor("maskT", [T, T], F32, kind="ExternalInput")
    y = nc.dram_tensor("y", [TO, C], F32, kind="ExternalOutput")

    def ktmax(qj):  # number of k-tiles for q-chunk qj
        return 4 * qj + 4 if mode == "causal" else NTT

    def wo_part_ap(hh, dj):
        r = (hh * NDJ + dj) * 128
        return woTt[r:r + 128, :]

    with tile.TileContext(nc) as tc, \
         tc.tile_pool(name="consts", bufs=1) as pc, \
         tc.tile_pool(name="dram", bufs=1, space="DRAM") as dram:
        ones_sb = pc.tile([128, 1], F32R)
        onesr_sb = pc.tile([1, 128], F32R)
        if mode == "causal":
            bm_sb = pc.tile([128, 2048], F32)

        def load_consts():
            nc.sync.dma_start(ones_sb[:], ones_in[:])
            nc.sync.dma_start(onesr_sb[:], onesr_in[:])
            if mode == "causal":
                nc.sync.dma_start(bm_sb[:], bmask[:])

        # AllToAll buffers, one pair per local head h: shard j (rows
        # j*HD..(j+1)*HD) holds this core's head-h outputs for dest
        # core j = (b*4 + t-slice). After A2A, cc_out[h] row block i =
        # core i's head h (channels i*D + h*HD ..), for THIS core's
        # (batch, t-slice). Per-head tensors let head h's A2A overlap
        # head h+1's attention.
        cc_in = [dram.tile([N_CORES * HD, TO], F32R, name=f"cc_in{h}")
                 for h in range(HPC)]
        cc_out = [dram.tile([N_CORES * HD, TO], F32R, name=f"cc_out{h}")
                  for h in range(HPC)]

        # qkv pool lives through attention, freed before o_proj
        with tc.tile_pool(name="qkv", bufs=1) as pq:
            # per (batch, head) pair p = b*HPC + h: q^T,k^T [hd, T]
            qT_sb = pq.tile([128, B * HPC * T], F32R)
            kT_sb = pq.tile([128, B * HPC * T], F32R)
            # v natural layout per batch: [T,128]-tile x [128, D]
            v_sb = pq.tile([128, B * NTT * D], F32R)

            # ============ Phase A: q,k,v projections + RoPE ============
            with tc.tile_pool(name="prj_w", bufs=1) as pw, \
                 tc.tile_pool(name="prj_x", bufs=2) as px, \
                 tc.tile_pool(name="prj_cs", bufs=2) as pcs, \
                 tc.tile_pool(name="prj_ps", bufs=8, space="PSUM") as pps, \
                 tc.tile_pool(name="prj_tmp", bufs=14) as pt:
                wq_sb = pw.tile([128, NCT * D], F32R)
                wk_sb = pw.tile([128, NCT * D], F32R)
                wv_sb = pw.tile([128, NCT * D], F32R)

                def load_w(wsb, wdr):
                    nc.sync.dma_start(wsb[:], wdr[:])

                def load_chunk(ch):
                    xch = px.tile([128, NCT * TCH], F32R, tag="xch",
                                  name="xch")
                    nc.sync.dma_start(
                        xch[:], xTt[ch * 128:(ch + 1) * 128, :])
                    tl = (ch * TCH) % T
                    cs = pcs.tile([128, TCH], F32, tag="cos", name="cs")
                    sn = pcs.tile([128, TCH], F32, tag="sin", name="sn")
                    nc.sync.dma_start(cs[:], cosT[:, tl:tl + TCH])
                    nc.sync.dma_start(sn[:], sinT[:, tl:tl + TCH])
                    return xch, cs, sn

                # interleave quarters of Wq and chunk-0 so the first
                # matmuls start after ~1MB instead of 4MB
                xch0 = px.tile([128, NCT * TCH], F32R, tag="xch",
                               name="xch0")
                QW = NCT * D // 4
                QX = NCT * TCH // 4
                for qtr in range(4):
                    nc.sync.dma_start(wq_sb[:, qtr * QW:(qtr + 1) * QW],
                                      wqT[:, qtr * QW:(qtr + 1) * QW])
                    nc.sync.dma_start(xch0[:, qtr * QX:(qtr + 1) * QX],
                                      xTt[0:128, qtr * QX:(qtr + 1) * QX])
                cs0 = pcs.tile([128, TCH], F32, tag="cos", name="cs0")
                sn0 = pcs.tile([128, TCH], F32, tag="sin", name="sn0")
                nc.sync.dma_start(cs0[:], cosT[:, 0:TCH])
                nc.sync.dma_start(sn0[:], sinT[:, 0:TCH])
                pre = (xch0, cs0, sn0)
                load_w(wk_sb, wkT)
                load_w(wv_sb, wvT)
                load_consts()

                for ch in range(NCH):
                    b = ch // (T // TCH)
                    tloc = (ch * TCH) % T
                    xch, cs, sn = pre if ch == 0 else load_chunk(ch)
                    pre = None
                    evac = []
                    for h in range(HPC):
                        p = b * HPC + h
                        for wsb, dst in ((wq_sb, qT_sb), (wk_sb, kT_sb)):
                            ps = pps.tile([128, TCH], F32, tag="pAqk",
                                          name="psA")
                            for ct in range(NCT):
                                nc.tensor.matmul(
                                    ps[:],
                                    wsb[:, ct * D + h * HD:
                                        ct * D + (h + 1) * HD],
                                    xch[:, ct * TCH:(ct + 1) * TCH],
                                    start=(ct == 0), stop=(ct == NCT - 1))
                            evac.append((ps, dst, p))
                    # pass 1: free the PSUM slots fast (t1 reads PSUM,
                    # the copy feeds the partition-swap DMA)
                    pass2 = []
                    for ps, dst, p in evac:
                        t1 = pt.tile([128, TCH], F32, tag="rope", name="t1")
                        nc.vector.tensor_mul(t1[:], ps[:], cs[:])
                        tmp = pt.tile([128, TCH], F32, tag="rope",
                                      name="rtmp")
                        nc.scalar.copy(tmp[:], ps[:])
                        sw = pt.tile([128, TCH], F32, tag="rope", name="rsw")
                        nc.gpsimd.dma_start(sw[0:64, :], tmp[64:128, :])
                        nc.gpsimd.dma_start(sw[64:128, :], tmp[0:64, :])
                        pass2.append((t1, sw, dst, p))
                    # v matmuls for this chunk (all local heads at once)
                    for st in range(TCH // 128):
                        tt = (ch * TCH) // 128 + st
                        ps = pps.tile([128, D], F32, tag="pAqk", name="psV")
                        for ct in range(NCT):
                            nc.tensor.matmul(
                                ps[:],
                                xch[:, ct * TCH + st * 128:
                                    ct * TCH + st * 128 + 128],
                                wv_sb[:, ct * D:(ct + 1) * D],
                                start=(ct == 0), stop=(ct == NCT - 1))
                        with nc.allow_low_precision(reason="f32r v evac"):
                            nc.scalar.copy(v_sb[:, tt * D:(tt + 1) * D],
                                           ps[:])
                    # pass 2: finish RoPE
                    for t1, sw, dst, p in pass2:
                        t2 = pt.tile([128, TCH], F32, tag="rope", name="t2")
                        nc.vector.tensor_mul(t2[:], sw[:], sn[:])
                        sl = dst[:, p * T + tloc: p * T + tloc + TCH]
                        with nc.allow_low_precision(reason="f32r rope"):
                            nc.vector.tensor_add(sl, t1[:], t2[:])

            # ============ Attention per (batch, head) =================
            # Software-pipelined emission: the S^T matmul for k-tile
            # kt+2 is issued before the den/PV matmuls of k-tile kt, so
            # the PE keeps streaming while ACT(exp)/DVE(mask) catch up.
            with tc.tile_pool(name="att_es", bufs=8) as pes, \
                 tc.tile_pool(name="att_o", bufs=3) as po, \
                 tc.tile_pool(name="att_ps", bufs=4, space="PSUM") as pas, \
                 tc.tile_pool(name="att_acc", bufs=2, space="PSUM") as paa, \
                 tc.tile_pool(name="att_msk", bufs=4) as pmk:
                for h in range(HPC):
                    for b in range(B):
                        p = b * HPC + h
                        for qj in range(NQC):
                            qsl = qT_sb[:, p * T + qj * 512:
                                        p * T + qj * 512 + 512]
                            kmax = ktmax(qj)
                            ps_den = paa.tile([1, 512], F32, tag="den",
                                              bufs=1, name="psden")
                            ps_o = paa.tile([128, 512], F32, tag="pvacc",
                                            bufs=3, name="pso")

                            def s_pair(kt2):
                                # two k-tiles share one 2-bank psum so a
                                # single ACT pass exponentiates both
                                s2 = pas.tile([128, 1024], F32, tag="s2",
                                              bufs=2, name="s2")
                                for half in range(2):
                                    kt = kt2 + half
                                    if kt >= kmax:
                                        break
                                    nc.tensor.matmul(
                                        s2[:, half * 512:half * 512 + 512],
                                        kT_sb[:, p * T + kt * 128:
                                              p * T + kt * 128 + 128],
                                        qsl, start=True, stop=True)
                                return s2

                            npair = (kmax + 1) // 2
                            s_tiles = {}
                            for kk in range(min(2, npair)):
                                s_tiles[kk] = s_pair(2 * kk)
                            for pi in range(npair):
                                s2 = s_tiles.pop(pi)
                                e2 = pes.tile([128, 1024], F32R, tag="es",
                                              name="e2")
                                if mode == "masked":
                                    sm2 = pmk.tile([128, 1024], F32,
                                                   tag="sm", name="sm2")
                                    mt2 = pmk.tile([128, 1024], F32,
                                                   tag="mt", name="mt2")
                                    nw = min(kmax - 2 * pi, 2)
                                    nc.sync.dma_start(
                                        mt2[:, :nw * 512].rearrange(
                                            "p (n q) -> p n q", n=nw),
                                        maskT[2 * pi * 128:
                                              (2 * pi + nw) * 128,
                                              qj * 512:(qj + 1) * 512]
                                        .rearrange("(n p) q -> p n q",
                                                   p=128))
                                    if nw < 2:
                                        nc.vector.memset(
                                            mt2[:, nw * 512:], 0.0)
                                    nc.vector.tensor_add(sm2[:], s2[:],
                                                         mt2[:])
                                    esrc = sm2
                                else:
                                    esrc = s2
                                with nc.allow_low_precision(reason="exp"):
                                    nc.scalar.activation(
                                        e2[:], esrc[:], AF.Exp,
                                        scale=float(scale))
                                if pi + 2 < npair:
                                    s_tiles[pi + 2] = s_pair(2 * (pi + 2))
                                if mode == "causal" and 2 * pi >= 4 * qj:
                                    v = (2 * pi - 4 * qj) // 2
                                    em2 = pes.tile([128, 1024], F32R,
                                                   tag="esm", bufs=4,
                                                   name="em2")
                                    with nc.allow_low_precision(
                                            reason="mask"):
                                        nc.vector.tensor_mul(
                                            em2[:], e2.bitcast(F32),
                                            bm_sb[:, v * 1024:
                                                  v * 1024 + 1024])
                                    esel = em2
                                else:
                                    esel = e2
                                for half in range(2):
                                    kt = 2 * pi + half
                                    if kt >= kmax:
                                        break
                                    e_t = esel[:, half * 512:half * 512 + 512]
                                    nc.tensor.matmul(
                                        ps_den[:], ones_sb[:], e_t,
                                        start=(kt == 0),
                                        stop=(kt == kmax - 1))
                                    nc.tensor.matmul(
                                        ps_o[:],
                                        v_sb[:, (b * NTT + kt) * D + h * HD:
                                             (b * NTT + kt) * D
                                             + (h + 1) * HD],
                                        e_t,
                                        start=(kt == 0),
                                        stop=(kt == kmax - 1))
                            # softmax denominator -> reciprocal, spread
                            # over partitions so DVE RECIPROCAL is wide
                            if True:
                                dsb = po.tile([1, 512], F32, tag="dsb",
                                              name="dsb")
                                nc.vector.tensor_copy(dsb[:], ps_den[:])
                                drs = po.tile([128, 4], F32, tag="drs",
                                              name="drs")
                                nc.sync.dma_start(drs[:], dsb[:])
                                rrs = po.tile([128, 4], F32, tag="rrs",
                                              name="rrs")
                                nc.vector.reciprocal(rrs[:], drs[:])
                                rsb = po.tile([1, 512], F32, tag="rsb",
                                              name="rsb")
                                nc.sync.dma_start(rsb[:], rrs[:])
                            bc = po.tile([128, 512], F32, tag="bc",
                                         name="bc")
                            nc.gpsimd.partition_broadcast(bc[:], rsb[:])
                            o_sc = po.tile([128, 512], F32R, tag="osc",
                                           name="osc")
                            with nc.allow_low_precision(reason="scale"):
                                nc.vector.tensor_mul(o_sc[:], ps_o[:],
                                                     bc[:])
                            # scatter the 512-wide q-chunk into shards
                            w = min(512, TO)
                            for s in range(512 // w):
                                t0 = qj * 512 + s * w    # global t in batch
                                shard = b * 4 + t0 // TO
                                nc.sync.dma_start(
                                    cc_in[h][shard * HD:(shard + 1) * HD,
                                             t0 % TO: t0 % TO + w],
                                    o_sc[:, s * w:(s + 1) * w])
                    # head h complete on both batches -> its AllToAll can
                    # overlap head h+1's attention
                    nc.gpsimd.collective_compute(
                        "AllToAll", mybir.AluOpType.bypass,
                        replica_groups=[list(range(N_CORES))],
                        ins=[cc_in[h].opt()], outs=[cc_out[h].opt()])

        # ============ Phase C: o_proj ================================
        # Heads 0..HPC-2 were gathered during attention, so their
        # o_proj contribution runs concurrently with the LAST head's
        # AllToAll (the PE is otherwise idle there); partials live in
        # SBUF. The last head's part joins when its A2A lands.
        with tc.tile_pool(name="phC_cc", bufs=1) as pcc, \
             tc.tile_pool(name="phC_w", bufs=3) as pcw, \
             tc.tile_pool(name="phC_y", bufs=4) as pcy, \
             tc.tile_pool(name="phC_y0", bufs=1) as pcy0, \
             tc.tile_pool(name="phC_ps", bufs=4, space="PSUM") as pcps:
            y0_sb = (pcy0.tile([128, (TO // 128) * C], F32, name="y0sb")
                     if HPC > 1 else None)
            cc_sb = {}

            def part(hh, first, last):
                t = pcc.tile([128, N_CORES * TO], F32R, name=f"cc_sb{hh}")
                nc.sync.dma_start(
                    t.rearrange("p (n t) -> p n t", n=N_CORES),
                    cc_out[hh].opt().rearrange("(n p) t -> p n t", p=128))
                cc_sb[hh] = t
                for dj in range(NDJ):
                    wo_sb = pcw.tile([128, NPR * DW], F32R, tag="wo",
                                     name="wo")
                    nc.sync.dma_start(
                        wo_sb.rearrange("p (n d) -> p n d", n=NPR),
                        wo_part_ap(hh, dj))
                    for tt in range(TO // 128):
                        ps = pcps.tile([128, DW], F32, tag="pC", name="psC")
                        for i in range(NPR):
                            nc.tensor.matmul(
                                ps[:],
                                cc_sb[hh][:, i * TO + tt * 128:
                                          i * TO + tt * 128 + 128],
                                wo_sb[:, i * DW:(i + 1) * DW],
                                start=(i == 0), stop=(i == NPR - 1))
                        y0sl = (y0_sb[:, tt * C + dj * DW:
                                      tt * C + (dj + 1) * DW]
                                if y0_sb is not None else None)
                        if last:
                            yt = pcy.tile([128, DW], F32, tag="yt",
                                          name="yt")
                            if first:
                                nc.scalar.copy(yt[:], ps[:])
                            else:
                                nc.vector.tensor_add(yt[:], ps[:], y0sl)
                            nc.sync.dma_start(
                                y[tt * 128:(tt + 1) * 128,
                                  dj * DW:(dj + 1) * DW], yt[:])
                        elif first:
                            nc.scalar.copy(y0sl, ps[:])
                        else:
                            nc.vector.tensor_add(y0sl, ps[:], y0sl)

            for hh in range(HPC):
                part(hh, first=(hh == 0), last=(hh == HPC - 1))

    nc.compile()
    return nc


_NC_CACHE = {}


def _get_nc(T, C, mode):
    key = (T, C, mode)
    if key not in _NC_CACHE:
        _NC_CACHE[key] = build_nc(T, C, mode)
    return _NC_CACHE[key]


def _detect_mode(mask):
    T = mask.shape[0]
    tri = np.tril(np.ones((T, T), dtype=bool))
    if not np.any(mask):
        return "full"
    if np.all(np.abs(mask[tri]) < 1e-6) and np.all(mask[~tri] < -1e8):
        return "causal"
    return "masked"


def kernel(x, mask, Wq, Wk, Wv, Wo):
    x = np.asarray(x)
    mask = np.asarray(mask)
    Bx, T, C = x.shape
    assert Bx == B
    HPC = C // HD // N_CORES
    TO = T // 4
    mode = _detect_mode(mask)
    nc = _get_nc(T, C, mode)

    cos, sin_signed = _rope_tables(T)
    NCT = C // 128
    TCH = 256
    NCH = B * T // TCH
    DW = 256
    NDJ = C // DW
    NPR = NCT // HPC
    D = HPC * HD
    xT2 = np.concatenate([x[0].T, x[1].T], axis=1)          # [C, B*T]
    # pre-tile x: [NCH*128, NCT*TCH]
    xtt = (xT2.reshape(NCT, 128, NCH, TCH).transpose(2, 1, 0, 3)
           .reshape(NCH * 128, NCT * TCH))
    xtt = np.ascontiguousarray(xtt)

    def tile_w(Wslice):     # [C, D] -> [128, NCT*D]
        wt = Wslice.T.reshape(NCT, 128, -1).transpose(1, 0, 2)
        return np.ascontiguousarray(wt.reshape(128, -1))

    WoT = np.asarray(Wo).T                                   # [C, C]
    # pre-tile wo: rows (hh*NDJ+dj)*128.. hold [128, NPR*DW] for the
    # NPR channel-tiles (i*HPC+hh) of d-chunk dj
    wott = np.empty((HPC * NDJ * 128, NPR * DW), np.float32)
    for hh in range(HPC):
        for dj in range(NDJ):
            blk = WoT.reshape(NPR, HPC, 128, C)[:, hh, :,
                                                dj * DW:(dj + 1) * DW]
            wott[(hh * NDJ + dj) * 128:(hh * NDJ + dj + 1) * 128, :] = \
                blk.transpose(1, 0, 2).reshape(128, NPR * DW)

    in_maps = []
    for core in range(N_CORES):
        hsl = slice(core * HPC * HD, (core + 1) * HPC * HD)
        m = {
            "xTt": xtt,
            "wqT": tile_w(np.asarray(Wq)[hsl, :]),
            "wkT": tile_w(np.asarray(Wk)[hsl, :]),
            "wvT": tile_w(np.asarray(Wv)[hsl, :]),
            "woTt": wott,
            "cosT": cos, "sinT": sin_signed,
            "ones_in": np.ones((128, 1), np.float32),
            "onesr_in": np.ones((1, 128), np.float32),
        }
        if mode == "causal":
            m["bmask"] = _causal_binmask()
        elif mode == "masked":
            m["maskT"] = np.ascontiguousarray(mask.T) * np.float32(np.sqrt(HD))
        in_maps.append(m)

    res = bass_utils.run_bass_kernel_spmd(nc, in_maps,
                                          core_ids=list(range(N_CORES)))

    out = np.empty((B, T, C), np.float32)
    for core in range(N_CORES):
        b, g = divmod(core, 4)
        out[b, g * TO:(g + 1) * TO, :] = res.results[core]["y"]
    return out
